# revision 6
# baseline (speedup 1.0000x reference)
"""Trainium2 Bass kernel for a local-attention transformer block.

Data-parallel over tokens: 8 shards of 1024 tokens + 128-token halo.
Restructured for PE streaming efficiency:
  - weight-stationary long-N GEMMs (QK N=512/384, FFN1 N=512) so
    LDWEIGHTS hides under the moving-operand stream
  - token-stationary N=384 GEMMs for V / out-proj / FFN2
  - attention computes S^T = K^T Q directly (row-packed K=64 matmul
    pairs), no P transposes; softmax denominator via a ones column
    appended to V; per-head normalization via ACT scale=AP
  - biases folded: per-partition ACT bias (QK, FFN1), rank-1 ones-row
    matmuls (V, out-proj, FFN2)
  - ACT table-set batching: exp -> sqrt(LN1) -> gelu/sqrt(LN2)
  - x kept in bf16; yT_all storage reused for x1T
"""

import numpy as np
import ml_dtypes
from contextlib import nullcontext as _nullctx

# ---- problem constants (hardcoded per contract) ----
B, S, D = 2, 4096, 768
NH, HD = 12, 64
DFF = 4 * D            # 3072
DQK = 2 * D            # 1536
WIN = 128
EPS = 1e-5
T = 128
NB = 8                 # own 128-token blocks per core
NBH = NB + 1           # with halo block
NTOK = NB * T          # 1024
NTOKH = NBH * T        # 1152
ND = D // T            # 6
NF = DFF // T          # 24
N_CORES = 8
NEG = -1e30

_CACHE = {}


def _build_nc(act="gelu", reps=1, loop=1):
    import concourse.bacc as bacc
    import concourse.mybir as mybir
    from concourse import tile
    from concourse.masks import make_identity
    from contextlib import ExitStack

    f32 = mybir.dt.float32
    bf16 = mybir.dt.bfloat16
    AF = mybir.ActivationFunctionType
    ALU = mybir.AluOpType

    nc = bacc.Bacc("TRN2", target_bir_lowering=False, debug=False,
                   num_devices=N_CORES)

    # ---- DRAM I/O ----
    xh_d = nc.dram_tensor("xh", [NTOKH, D], bf16, kind="ExternalInput").ap()
    mf_d = nc.dram_tensor("mask_first", [T, 2, T], bf16, kind="ExternalInput").ap()
    mr_d = nc.dram_tensor("mask_rest", [T, 2, T], bf16, kind="ExternalInput").ap()
    wqk_d = nc.dram_tensor("wqkT", [D, DQK], bf16, kind="ExternalInput").ap()
    wv_d = nc.dram_tensor("wvT", [D, D], bf16, kind="ExternalInput").ap()
    wo_d = nc.dram_tensor("woT", [D, D], bf16, kind="ExternalInput").ap()
    w1_d = nc.dram_tensor("w1T", [D, DFF], bf16, kind="ExternalInput").ap()
    w2_d = nc.dram_tensor("w2T", [DFF, D], bf16, kind="ExternalInput").ap()
    qkb_d = nc.dram_tensor("qkb", [T, 12], f32, kind="ExternalInput").ap()
    b1c_d = nc.dram_tensor("b1c", [T, NF], f32, kind="ExternalInput").ap()
    vb_d = nc.dram_tensor("vb_row", [1, D], bf16, kind="ExternalInput").ap()
    ob_d = nc.dram_tensor("ob_row", [1, D], bf16, kind="ExternalInput").ap()
    b2_d = nc.dram_tensor("b2_row", [1, D], bf16, kind="ExternalInput").ap()
    g1_d = nc.dram_tensor("g1_rep", [T, D], bf16, kind="ExternalInput").ap()
    bb1_d = nc.dram_tensor("b1_rep", [T, D], bf16, kind="ExternalInput").ap()
    g2_d = nc.dram_tensor("g2_rep", [T, D], bf16, kind="ExternalInput").ap()
    bb2_d = nc.dram_tensor("b2_rep", [T, D], bf16, kind="ExternalInput").ap()
    out_d = nc.dram_tensor("out", [NTOK, D], f32, kind="ExternalOutput").ap()

    with tile.TileContext(nc) as tc:
      with (tc.For_i(0, loop, 1) if loop > 1 else _nullctx()):
        for rep in range(reps):
          top = ExitStack()
          persist = top.enter_context(tc.tile_pool(name=f"persist{rep}", bufs=1))
          ident = persist.tile([T, T], bf16, tag="ident")
          make_identity(nc, ident[:])
          mf_sb = persist.tile([T, 2, T], bf16, tag="mf")
          nc.sync.dma_start(mf_sb[:], mf_d[:])
          mr_sb = persist.tile([T, 2, T], bf16, tag="mr")
          nc.sync.dma_start(mr_sb[:], mr_d[:])
          qkb_sb = persist.tile([T, 12], f32, tag="qkb")
          nc.sync.dma_start(qkb_sb[:], qkb_d[:])
          b1c_sb = persist.tile([T, NF], f32, tag="b1c")
          nc.sync.dma_start(b1c_sb[:], b1c_d[:])
          eps_sb = persist.tile([T, 1], f32, tag="eps")
          nc.gpsimd.memset(eps_sb[:], EPS)
          ones_sb = persist.tile([1, T], bf16, tag="ones")
          nc.gpsimd.memset(ones_sb[:], 1.0)
          vb_sb = persist.tile([1, D], bf16, tag="vb")
          nc.sync.dma_start(vb_sb[:], vb_d[:])
          ob_sb = persist.tile([1, D], bf16, tag="ob")
          nc.sync.dma_start(ob_sb[:], ob_d[:])
          b2r_sb = persist.tile([1, D], bf16, tag="b2r")
          nc.sync.dma_start(b2r_sb[:], b2_d[:])
          g1_sb = persist.tile([T, D], bf16, tag="g1")
          nc.sync.dma_start(g1_sb[:], g1_d[:])
          bb1_sb = persist.tile([T, D], bf16, tag="bb1")
          nc.sync.dma_start(bb1_sb[:], bb1_d[:])
          g2_sb = persist.tile([T, D], bf16, tag="g2")
          nc.sync.dma_start(g2_sb[:], g2_d[:])
          bb2_sb = persist.tile([T, D], bf16, tag="bb2")
          nc.sync.dma_start(bb2_sb[:], bb2_d[:])

          # wo early so out-proj never waits on its DMA
          wo_sb = persist.tile([T, ND, D], bf16, tag="wo")
          nc.sync.dma_start(wo_sb[:], wo_d.rearrange("(j p) n -> p j n", p=T))

          x_stack = ExitStack()
          xpool = x_stack.enter_context(tc.tile_pool(name=f"x{rep}", bufs=1))
          x_sb = xpool.tile([T, NBH, D], bf16, tag="x")
          nc.sync.dma_start(x_sb[:], xh_d.rearrange("(i p) d -> p i d", p=T))
          # yT storage; later reused as x1T (same shape/dtype)
          tT_all = xpool.tile([T, NB, ND, T], bf16, tag="tT")

          qkv_stack = ExitStack()
          qkvp = qkv_stack.enter_context(tc.tile_pool(name=f"qkv{rep}", bufs=1))
          qT_sb = qkvp.tile([T, ND, NTOK], bf16, tag="qT")
          kT_sb = qkvp.tile([T, ND, NTOKH], bf16, tag="kT")
          v_sb = qkvp.tile([T, NBH, NH, HD + 1], bf16, tag="v")
          nc.vector.memset(v_sb[:, :, :, HD:HD + 1], 1.0)

          # ============ phase A: xT, QK (weight-stationary), V ==============
          with tc.tile_pool(name=f"wA{rep}", bufs=1) as wA, \
               tc.tile_pool(name=f"xT{rep}", bufs=1) as xTp, \
               tc.tile_pool(name=f"psA{rep}", bufs=2, space="PSUM") as psA, \
               tc.tile_pool(name=f"psTr{rep}", bufs=2, space="PSUM") as psTr:
              wqk_sb = wA.tile([T, ND, DQK], bf16, tag="wqk")
              nc.sync.dma_start(wqk_sb[:], wqk_d.rearrange("(j p) n -> p j n", p=T))
              wv_sb = wA.tile([T, ND, D], bf16, tag="wv")
              nc.sync.dma_start(wv_sb[:], wv_d.rearrange("(j p) n -> p j n", p=T))
              xT_all = xTp.tile([T, ND, NTOKH], bf16, tag="xT")

              for i in range(NBH):
                  for j in range(ND):
                      ptr = psTr.tile([T, T], bf16, tag="tr")
                      nc.tensor.transpose(ptr[:], x_sb[:, i, j * T:(j + 1) * T],
                                          ident[:])
                      nc.vector.tensor_copy(xT_all[:, j, i * T:(i + 1) * T],
                                            ptr[:])

              # Q: own tokens only (halo-coord 128..1152), 2 chunks of 512
              for ci in range(ND):
                  for tg in range(2):
                      sl = slice(T + tg * 512, T + (tg + 1) * 512)
                      pq = psA.tile([T, 512], f32, tag="pqk")
                      for j in range(ND):
                          nc.tensor.matmul(pq[:],
                                           wqk_sb[:, j, ci * T:(ci + 1) * T],
                                           xT_all[:, j, sl],
                                           start=(j == 0), stop=(j == ND - 1))
                      nc.scalar.activation(qT_sb[:, ci, tg * 512:(tg + 1) * 512],
                                           pq[:], AF.Identity,
                                           bias=qkb_sb[:, ci:ci + 1])
              # K: all 1152 tokens, 3 chunks of 384
              for ci in range(ND):
                  for tg in range(3):
                      sl = slice(tg * 384, (tg + 1) * 384)
                      pk = psA.tile([T, 384], f32, tag="pqk")
                      for j in range(ND):
                          nc.tensor.matmul(pk[:],
                                           wqk_sb[:, j, (ND + ci) * T:(ND + ci + 1) * T],
                                           xT_all[:, j, sl],
                                           start=(j == 0), stop=(j == ND - 1))
                      nc.scalar.activation(kT_sb[:, ci, sl], pk[:], AF.Identity,
                                           bias=qkb_sb[:, ND + ci:ND + ci + 1])
              # V: token-stationary, 2 halves of 384 channels + ones-row bias
              for i in range(NBH):
                  for nh in range(2):
                      sl = slice(nh * 384, (nh + 1) * 384)
                      pv = psA.tile([T, 6, HD], f32, tag="pv")
                      for j in range(ND):
                          nc.tensor.matmul(pv[:], xT_all[:, j, i * T:(i + 1) * T],
                                           wv_sb[:, j, sl],
                                           start=(j == 0), stop=False)
                      nc.tensor.matmul(pv[:], ones_sb[0:1, :], vb_sb[0:1, sl],
                                       start=False, stop=True)
                      nc.vector.tensor_copy(
                          v_sb[:, i, 6 * nh:6 * nh + 6, 0:HD], pv[:])

          # ============ phase B: attention (S^T layout) =====================
          with tc.tile_pool(name=f"attn{rep}", bufs=3) as attnp, \
               tc.tile_pool(name=f"yblk{rep}", bufs=2) as yblkp, \
               tc.tile_pool(name=f"psS{rep}", bufs=3, space="PSUM") as psS, \
               tc.tile_pool(name=f"psY{rep}", bufs=2, space="PSUM") as psY, \
               tc.tile_pool(name=f"psT2{rep}", bufs=2, space="PSUM") as psT2:
              for t in range(NB):
                  msk = mf_sb if t == 0 else mr_sb
                  y_blk = yblkp.tile([T, D], bf16, tag="yb")
                  for h in range(NH):
                      ci, s = h // 2, h % 2
                      po = s * HD
                      s_ps = psS.tile([T, 2, T], f32, tag="s")
                      for bb in range(2):
                          nc.tensor.matmul(
                              s_ps[:, bb, :],
                              kT_sb[po:po + HD, ci, (t + bb) * T:(t + bb + 1) * T],
                              qT_sb[po:po + HD, ci, t * T:(t + 1) * T],
                              start=True, stop=True)
                      nc.vector.tensor_tensor(s_ps[:], s_ps[:], msk[:],
                                              op=ALU.add)
                      pt = attnp.tile([T, 2, T], bf16, tag="pt")
                      nc.scalar.activation(pt[:], s_ps[:], AF.Exp, scale=0.125)
                      y_ps = psY.tile([T, HD + 1], f32, tag="y")
                      for bb in range(2):
                          nc.tensor.matmul(y_ps[:], pt[:, bb, :],
                                           v_sb[:, t + bb, h, :],
                                           start=(bb == 0), stop=(bb == 1))
                      rec = attnp.tile([T, 1], f32, tag="rec")
                      nc.vector.reciprocal(rec[:], y_ps[:, HD:HD + 1])
                      nc.scalar.mul(y_blk[:, h * HD:(h + 1) * HD],
                                    y_ps[:, 0:HD], rec[:])
                  for j in range(ND):
                      ptr = psT2.tile([T, T], bf16, tag="ytr")
                      nc.tensor.transpose(ptr[:], y_blk[:, j * T:(j + 1) * T],
                                          ident[:])
                      nc.vector.tensor_copy(tT_all[:, t, j, :], ptr[:])

          qkv_stack.close()

          # ============ phase C: out-proj + LN1 (+x1T into tT_all) ==========
          w12_stack = ExitStack()
          w12p = w12_stack.enter_context(tc.tile_pool(name=f"w12{rep}", bufs=1))
          w1_sb = w12p.tile([T, ND, DFF], bf16, tag="w1")
          nc.sync.dma_start(w1_sb[:], w1_d.rearrange("(j p) n -> p j n", p=T))
          w2_sb = w12p.tile([T, NF, D], bf16, tag="w2")
          nc.sync.dma_start(w2_sb[:], w2_d.rearrange("(j p) n -> p j n", p=T))

          x1_stack = ExitStack()
          x1p = x1_stack.enter_context(tc.tile_pool(name=f"x1{rep}", bufs=1))
          x1_sb = x1p.tile([T, NB, D], bf16, tag="x1")

          def emit_ln(pool, xpre, s0, s1, g_rep, b_rep, out_ap):
              ns = pool.tile([T, 1], f32, tag="ln_ns")
              nc.vector.tensor_tensor(ns[:], s0, s1, op=ALU.add)
              nm = pool.tile([T, 1], f32, tag="ln_nm")
              nc.scalar.mul(nm[:], ns[:], -1.0 / D)
              xc = pool.tile([T, D], bf16, tag="ln_xc")
              nc.vector.tensor_scalar_add(xc[:], xpre[:], nm[:])
              sq = pool.tile([T, D], bf16, tag="ln_sq")
              vs = pool.tile([T, 1], f32, tag="ln_vs")
              nc.vector.scalar_tensor_tensor(sq[:], xc[:], 1.0, xc[:],
                                             op0=ALU.mult, op1=ALU.mult,
                                             accum_out=vs[:])
              std = pool.tile([T, 1], f32, tag="ln_std")
              nc.scalar.activation(std[:], vs[:], AF.Sqrt, bias=eps_sb[:],
                                   scale=1.0 / D)
              rstd = pool.tile([T, 1], f32, tag="ln_rstd")
              nc.vector.reciprocal(rstd[:], std[:])
              xg = pool.tile([T, D], bf16, tag="ln_xg")
              nc.vector.scalar_tensor_tensor(xg[:], xc[:], rstd[:], g_rep[:],
                                             op0=ALU.mult, op1=ALU.mult)
              nc.vector.tensor_tensor(out_ap, xg[:], b_rep[:], op=ALU.add)

          with tc.tile_pool(name=f"workC{rep}", bufs=2) as workC, \
               tc.tile_pool(name=f"psC{rep}", bufs=2, space="PSUM") as psC, \
               tc.tile_pool(name=f"psT3{rep}", bufs=2, space="PSUM") as psT3:
              for t in range(NB):
                  x1pre = workC.tile([T, D], f32, tag="x1pre")
                  ssum = workC.tile([T, 2], f32, tag="ssum")
                  for nh in range(2):
                      sl = slice(nh * 384, (nh + 1) * 384)
                      pz = psC.tile([T, 384], f32, tag="mm")
                      for j in range(ND):
                          nc.tensor.matmul(pz[:], tT_all[:, t, j, :],
                                           wo_sb[:, j, sl],
                                           start=(j == 0), stop=False)
                      nc.tensor.matmul(pz[:], ones_sb[0:1, :], ob_sb[0:1, sl],
                                       start=False, stop=True)
                      nc.vector.scalar_tensor_tensor(
                          x1pre[:, sl], pz[:], 1.0, x_sb[:, t + 1, sl],
                          op0=ALU.mult, op1=ALU.add,
                          accum_out=ssum[:, nh:nh + 1])
                  emit_ln(workC, x1pre, ssum[:, 0:1], ssum[:, 1:2],
                          g1_sb, bb1_sb, x1_sb[:, t, :])
                  # x1T overwrites this block's yT slots (already consumed)
                  for j in range(ND):
                      ptr = psT3.tile([T, T], bf16, tag="x1tr")
                      nc.tensor.transpose(ptr[:], x1_sb[:, t, j * T:(j + 1) * T],
                                          ident[:])
                      nc.vector.tensor_copy(tT_all[:, t, j, :], ptr[:])

          # ============ phase D: FFN + LN2 ==================================
          act_fn = AF.Gelu if act == "gelu" else AF.Identity
          with tc.tile_pool(name=f"h{rep}", bufs=1) as hp, \
               tc.tile_pool(name=f"workD{rep}", bufs=2) as workD, \
               tc.tile_pool(name=f"psD1{rep}", bufs=3, space="PSUM") as psD1, \
               tc.tile_pool(name=f"psD2{rep}", bufs=2, space="PSUM") as psD2:
              for half in range(2):
                  h_sb = hp.tile([T, NF, 512], bf16, tag="h")
                  for fi in range(NF):
                      ph = psD1.tile([T, 512], f32, tag="h1")
                      for j in range(ND):
                          nc.tensor.matmul(
                              ph[:], w1_sb[:, j, fi * T:(fi + 1) * T],
                              tT_all[:, half * 4:half * 4 + 4, j, :],
                              start=(j == 0), stop=(j == ND - 1))
                      nc.scalar.activation(h_sb[:, fi, :], ph[:], act_fn,
                                           bias=b1c_sb[:, fi:fi + 1])
                  for tq in range(4):
                      t = half * 4 + tq
                      x2pre = workD.tile([T, D], bf16, tag="x2pre")
                      ssum2 = workD.tile([T, 2], f32, tag="ssum2")
                      for nh in range(2):
                          sl = slice(nh * 384, (nh + 1) * 384)
                          pz2 = psD2.tile([T, 384], f32, tag="mm2")
                          for fi in range(NF):
                              nc.tensor.matmul(pz2[:],
                                               h_sb[:, fi, tq * T:(tq + 1) * T],
                                               w2_sb[:, fi, sl],
                                               start=(fi == 0), stop=False)
                          nc.tensor.matmul(pz2[:], ones_sb[0:1, :],
                                           b2r_sb[0:1, sl],
                                           start=False, stop=True)
                          nc.vector.scalar_tensor_tensor(
                              x2pre[:, sl], pz2[:], 1.0, x1_sb[:, t, sl],
                              op0=ALU.mult, op1=ALU.add,
                              accum_out=ssum2[:, nh:nh + 1])
                      out_sb = workD.tile([T, D], f32, tag="outb")
                      emit_ln(workD, x2pre, ssum2[:, 0:1], ssum2[:, 1:2],
                              g2_sb, bb2_sb, out_sb[:])
                      nc.sync.dma_start(out_d[t * T:(t + 1) * T, :], out_sb[:])

          x1_stack.close()
          w12_stack.close()
          x_stack.close()
          top.close()

    nc.compile()
    return nc


def _get_nc(act="gelu", reps=1, loop=1):
    key = (act, reps, loop)
    if key not in _CACHE:
        _CACHE[key] = _build_nc(act, reps, loop)
    return _CACHE[key]


def make_in_maps(x, in_proj_w, in_proj_b, out_w, out_b, ff_w1, ff_b1,
                 ff_w2, ff_b2, n1_g, n1_b, n2_g, n2_b):
    bf = ml_dtypes.bfloat16
    f32 = np.float32
    x = np.asarray(x, f32).reshape(B, S, D)

    shared = {
        "wqkT": np.ascontiguousarray(np.asarray(in_proj_w, f32)[:DQK].T).astype(bf),
        "wvT": np.ascontiguousarray(np.asarray(in_proj_w, f32)[DQK:].T).astype(bf),
        "woT": np.ascontiguousarray(np.asarray(out_w, f32).T).astype(bf),
        "w1T": np.ascontiguousarray(np.asarray(ff_w1, f32).T).astype(bf),
        "w2T": np.ascontiguousarray(np.asarray(ff_w2, f32).T).astype(bf),
        "qkb": np.ascontiguousarray(
            np.asarray(in_proj_b, f32)[:DQK].reshape(12, T).T),
        "b1c": np.ascontiguousarray(np.asarray(ff_b1, f32).reshape(NF, T).T),
        "vb_row": np.asarray(in_proj_b, f32)[DQK:].reshape(1, D).astype(bf),
        "ob_row": np.asarray(out_b, f32).reshape(1, D).astype(bf),
        "b2_row": np.asarray(ff_b2, f32).reshape(1, D).astype(bf),
        "g1_rep": np.ascontiguousarray(
            np.broadcast_to(np.asarray(n1_g, f32)[None, :], (T, D))).astype(bf),
        "b1_rep": np.ascontiguousarray(
            np.broadcast_to(np.asarray(n1_b, f32)[None, :], (T, D))).astype(bf),
        "g2_rep": np.ascontiguousarray(
            np.broadcast_to(np.asarray(n2_g, f32)[None, :], (T, D))).astype(bf),
        "b2_rep": np.ascontiguousarray(
            np.broadcast_to(np.asarray(n2_b, f32)[None, :], (T, D))).astype(bf),
    }

    # transposed-score masks: [key j (partition), block, query i (free)]
    j = np.arange(T, dtype=np.int64)[:, None]
    i = np.arange(T, dtype=np.int64)[None, :]
    M0T = np.where(j > i, 0.0, NEG).astype(f32)   # prev key block
    M1T = np.where(j <= i, 0.0, NEG).astype(f32)  # diagonal key block
    mask_rest = np.ascontiguousarray(np.stack([M0T, M1T], axis=1)).astype(bf)
    mask_first_bs = np.ascontiguousarray(
        np.stack([np.full((T, T), NEG, f32), M1T], axis=1)).astype(bf)

    in_maps = []
    for c in range(N_CORES):
        b, i0 = divmod(c * NTOK, S)
        halo = (np.zeros((T, D), f32) if i0 == 0
                else x[b, i0 - T:i0])
        xh = np.ascontiguousarray(
            np.concatenate([halo, x[b, i0:i0 + NTOK]], axis=0)).astype(bf)
        m = dict(shared)
        m["xh"] = xh
        m["mask_first"] = mask_first_bs if i0 == 0 else mask_rest
        m["mask_rest"] = mask_rest
        in_maps.append(m)
    return in_maps


def kernel(**inputs):
    from concourse.bass_utils import run_bass_kernel_spmd
    nc = _get_nc()
    in_maps = make_in_maps(**inputs)
    res = run_bass_kernel_spmd(nc, in_maps, core_ids=list(range(N_CORES)))
    outs = [res.results[c]["out"] for c in range(N_CORES)]
    return np.concatenate(outs, axis=0).reshape(B, S, D).astype(np.float32)
